# revision 25
# baseline (speedup 1.0000x reference)
"""Trainium2 Bass kernel for nn_Attention3 (dense multi-scale attention).

Sharding: pure data-parallel over batch — B=8, one batch element per
NeuronCore.  Each core runs the full per-batch computation; no collectives.

Per-core pipeline (all matmuls bf16 with fp32 PSUM accumulation):
  1. LayerNorm x0/x1/x2 in natural layout (DVE bn_stats/bn_aggr, ACT ln/exp
     for rsqrt), writing normalized x-hat as bf16 to DRAM scratch.
  2. xbar DMA-transpose x-hat back to SBUF in [d, n] layout.
  3. Projections q0^T / k^T / v / q2^T on PE (LN gamma/scale folded into
     weights, beta folded into per-channel bias columns).
  4. Attention in transposed layout: scores^T = k^T-vs-q^T matmuls,
     exp on ACT straight out of PSUM, o^T = v'-vs-exp^T matmuls where v'
     carries an extra all-ones column producing the softmax denominator.
  5. Normalize via reciprocal + DRAM-roundtrip row broadcast, assemble
     concat outs^T via partition-shifting SBUF DMAs, final out = outs @ Wout.

Fine-branch queries are loaded in (n1, n)-permuted order so the 4-token
channel fold becomes contiguous block copies.
"""
import os
import sys

sys.path.insert(0, "/opt/trn_rl_repo")

import numpy as np

DEBUG = bool(int(os.environ.get("K_DEBUG", "0")))

import concourse.bass as bass
import concourse.mybir as mybir
import concourse.tile as tile
from concourse import bacc, bass_utils

F32 = mybir.dt.float32
BF16 = mybir.dt.bfloat16
AF = mybir.ActivationFunctionType
ALU = mybir.AluOpType

B = 8
NK = 1024          # kv tokens
N0 = 4 * NK        # fine query tokens (4096)
N2 = NK // 4       # coarse query tokens (256)
D0, D1, D2 = 256, 512, 1024
DH = 64
H0, H2 = 2, 8
E = 512            # output dim
CD = 640           # concat dim
EPS = 1e-5
SCALE = DH ** -0.5


def _emit(nc, tc, t):
    """Emit the whole per-core kernel. t = dict of dram tensor handles."""
    import contextlib
    ctx = contextlib.ExitStack()

    sb = ctx.enter_context(tc.tile_pool(name="sb", bufs=1))
    ln = ctx.enter_context(tc.tile_pool(name="ln", bufs=2))
    wtmp = ctx.enter_context(tc.tile_pool(name="wtmp", bufs=2))
    expp = ctx.enter_context(tc.tile_pool(name="expp", bufs=2))
    orawp = ctx.enter_context(tc.tile_pool(name="orawp", bufs=2))
    stg = ctx.enter_context(tc.tile_pool(name="stg", bufs=2))
    outp = ctx.enter_context(tc.tile_pool(name="outp", bufs=2))
    ps_sc = ctx.enter_context(tc.tile_pool(name="ps_sc", bufs=2, space="PSUM"))
    ps_o = ctx.enter_context(tc.tile_pool(name="ps_o", bufs=2, space="PSUM"))
    ps_pr = ctx.enter_context(tc.tile_pool(name="ps_pr", bufs=2, space="PSUM"))
    dram = ctx.enter_context(tc.tile_pool(name="dram", bufs=1, space="DRAM"))

    # ---------------- weights prep ----------------
    # W stored as [128, S, Eout] with contraction dim striped over partitions.
    Wq0f = sb.tile([128, 2, 128], F32)
    nc.sync.dma_start(Wq0f[:], t["Wq0"][:].rearrange("(s p) e -> p s e", p=128))
    Wkvf = sb.tile([128, 4, 128], F32)
    nc.sync.dma_start(Wkvf[:], t["Wkv"][:].rearrange("(s p) e -> p s e", p=128))
    Woutbf = sb.tile([128, 5, 512], BF16)
    nc.gpsimd.dma_start(Woutbf[:], t["Wout"][:].rearrange("(s p) e -> p s e", p=128))

    g0 = sb.tile([128, 2], F32)
    nc.sync.dma_start(g0[:], t["g0"][:].rearrange("(s p) -> p s", p=128))
    b0 = sb.tile([128, 2], F32)
    nc.sync.dma_start(b0[:], t["b0"][:].rearrange("(s p) -> p s", p=128))
    g1 = sb.tile([128, 4], F32)
    nc.sync.dma_start(g1[:], t["g1"][:].rearrange("(s p) -> p s", p=128))
    b1 = sb.tile([128, 4], F32)
    nc.sync.dma_start(b1[:], t["b1"][:].rearrange("(s p) -> p s", p=128))
    g2 = sb.tile([128, 8], F32)
    nc.sync.dma_start(g2[:], t["g2"][:].rearrange("(s p) -> p s", p=128))
    b2 = sb.tile([128, 8], F32)
    nc.sync.dma_start(b2[:], t["b2"][:].rearrange("(s p) -> p s", p=128))

    # bias columns t^T = (b @ W)^T, computed before gamma folding
    tq0T = sb.tile([128, 1], F32)
    ps = ps_pr.tile([128, 512], F32)
    for s in range(2):
        nc.tensor.matmul(ps[:, 0:1], Wq0f[:, s, :], b0[:, s, None],
                         start=(s == 0), stop=(s == 1))
    nc.vector.tensor_scalar_mul(tq0T[:], ps[:, 0:1], SCALE)

    tkvT = sb.tile([128, 1], F32)
    ps = ps_pr.tile([128, 512], F32)
    for s in range(4):
        nc.tensor.matmul(ps[:, 0:1], Wkvf[:, s, :], b1[:, s, None],
                         start=(s == 0), stop=(s == 3))
    nc.vector.tensor_copy(tkvT[:], ps[:, 0:1])

    # t_v as a broadcast row: bias for v channels, [1,64] -> DRAM -> [128,64]
    tvrow = sb.tile([1, 64], F32)
    ps = ps_pr.tile([128, 512], F32)
    for s in range(4):
        nc.tensor.matmul(ps[0:1, 0:64], b1[:, s, None], Wkvf[:, s, 64:128],
                         start=(s == 0), stop=(s == 3))
    nc.vector.tensor_copy(tvrow[:], ps[0:1, 0:64])
    tv_dram = dram.tile([1, 64], F32)
    nc.sync.dma_start(tv_dram[:], tvrow[:])
    tvb = sb.tile([128, 64], F32)
    nc.sync.dma_start(
        tvb[:],
        bass.AP(tensor=tv_dram.tensor, offset=tv_dram[:].offset,
                ap=[[0, 128]] + list(tv_dram[:].ap[1:])),
    )

    # gamma (+ attention scale) folded into bf16 weights
    Wq0b = sb.tile([128, 2, 128], BF16)
    for s in range(2):
        nc.vector.tensor_scalar(out=Wq0b[:, s, :], in0=Wq0f[:, s, :],
                                scalar1=g0[:, s, None], scalar2=SCALE,
                                op0=ALU.mult, op1=ALU.mult)
    Wkvb = sb.tile([128, 4, 128], BF16)
    for s in range(4):
        nc.vector.tensor_scalar(out=Wkvb[:, s, :], in0=Wkvf[:, s, :],
                                scalar1=g1[:, s, None], scalar2=1.0,
                                op0=ALU.mult, op1=ALU.mult)

    # Wq2 streamed per K-slice (16KB/partition is too much to keep in f32).
    # t_q2 computed in row form [1, 512] (single PSUM group), then converted
    # to column form [128, 4] via a DRAM roundtrip.
    Wq2b = sb.tile([128, 8, 512], BF16)
    ps_b = ps_pr.tile([128, 512], F32, tag="ps", name="ps_b")
    for s in range(8):
        wq2s = wtmp.tile([128, 512], F32)
        nc.sync.dma_start(
            wq2s[:], t["Wq2"][:].rearrange("(s p) e -> p s e", p=128)[:, s, :])
        nc.tensor.matmul(ps_b[0:1, :], b2[:, s, None], wq2s[:],
                         start=(s == 0), stop=(s == 7))
        nc.vector.tensor_scalar(out=Wq2b[:, s, :], in0=wq2s[:],
                                scalar1=g2[:, s, None], scalar2=SCALE,
                                op0=ALU.mult, op1=ALU.mult)
    tq2row = sb.tile([1, 512], F32)
    nc.vector.tensor_scalar_mul(tq2row[:], ps_b[0:1, :], SCALE)
    tq2_dram = dram.tile([1, 512], F32)
    nc.sync.dma_start(tq2_dram[:], tq2row[:])
    tq2T = sb.tile([128, 4], F32)
    nc.sync.dma_start(tq2T[:], tq2_dram[0, :].rearrange("(mt p) -> p mt", p=128))

    # ---------------- LayerNorm + transpose ----------------
    eps_t = sb.tile([128, 1], F32)
    nc.vector.memset(eps_t[:], EPS)

    def emit_ln(name, xap, n_tokens, d, perm):
        """LN in natural layout -> bf16 x-hat in DRAM -> transposed SBUF."""
        ntiles = n_tokens // 128
        nsub = max(1, d // 512)
        tile_groups = max(1, 2048 // d)  # tiles per group (group = 2048 f32)
        ngroups = ntiles // tile_groups
        xhat_d = dram.tile([n_tokens, d], BF16)
        if perm:
            src = xap.rearrange("(n n1) d -> n1 n d", n1=4)
        else:
            src = xap.rearrange("(g n) d -> g n d", g=ngroups)
        for g in range(ngroups):
            xf = ln.tile([128, tile_groups, d], F32, tag="lnxf")
            # one DMA per group; for perm the rows are strided by 4
            gsrc = src[g].rearrange("(tl p) d -> p tl d", p=128)
            nc.sync.dma_start(xf[:], gsrc)
            stats = ln.tile([128, tile_groups, nsub, 6], F32, tag="lnstats")
            mv = ln.tile([128, tile_groups, 2], F32, tag="lnmv")
            for i in range(tile_groups):
                for k in range(nsub):
                    nc.vector.bn_stats(stats[:, i, k, :],
                                       xf[:, i, k * 512:(k + 1) * 512] if nsub > 1
                                       else xf[:, i, :])
                nc.vector.bn_aggr(mv[:, i, :], stats[:, i, :, :])
            rs = ln.tile([128, tile_groups], F32, tag="lnrs")
            nc.scalar.activation(out=rs[:], in_=mv[:, :, 1],
                                 func=AF.Abs_reciprocal_sqrt,
                                 bias=eps_t[:], scale=1.0)
            xh = ln.tile([128, tile_groups, d], BF16, tag="lnxh")
            for i in range(tile_groups):
                nc.vector.tensor_scalar(out=xh[:, i, :], in0=xf[:, i, :],
                                        scalar1=mv[:, i, 0:1],
                                        scalar2=rs[:, i, None],
                                        op0=ALU.subtract, op1=ALU.mult)
            dst = xhat_d[:].rearrange("(g tl p) d -> g p tl d", g=ngroups, p=128)
            nc.sync.dma_start(dst[g], xh[:])
        # transpose: [n, d] -> [d, n] as [128, d/128, n]
        xT = sb.tile([128, d // 128, n_tokens], BF16, tag=f"xT_{name}")
        for s in range(d // 128):
            nc.sync.dma_start_transpose(xT[:, s, :],
                                        xhat_d[:, s * 128:(s + 1) * 128])
        return xT

    x1T = emit_ln("x1", t["x1"][:], NK, D1, perm=False)
    x0T = emit_ln("x0", t["x0"][:], N0, D0, perm=True)
    x2T = emit_ln("x2", t["x2"][:], N2, D2, perm=False)
    if DEBUG:
        nc.sync.dma_start(t["dbg_x0T"][:], x0T[:])
        nc.sync.dma_start(t["dbg_x1T"][:], x1T[:])

    # ---------------- projections ----------------
    # k^T duplicated on both partition halves [128, 1024] for row-packed
    # head-pair score matmuls (rows 0-63 == rows 64-127 == k^T)
    kTd = sb.tile([128, NK], BF16)
    for half in range(2):
        ps = ps_pr.tile([128, 512], F32)
        for s in range(4):
            nc.tensor.matmul(ps[0:64, :], Wkvb[:, s, 0:64],
                             x1T[:, s, half * 512:(half + 1) * 512],
                             start=(s == 0), stop=(s == 3))
        nc.vector.tensor_scalar(out=kTd[0:64, half * 512:(half + 1) * 512],
                                in0=ps[0:64, :], scalar1=tkvT[0:64, :],
                                scalar2=0.0, op0=ALU.add, op1=ALU.add)
        nc.vector.tensor_scalar(out=kTd[64:128, half * 512:(half + 1) * 512],
                                in0=ps[0:64, :], scalar1=tkvT[0:64, :],
                                scalar2=0.0, op0=ALU.add, op1=ALU.add)

    # v' natural [128(t), 8, 65]: 64 v channels + ones column (denominator)
    vp = sb.tile([128, 8, 65], BF16)
    nc.vector.memset(vp[:, :, 64:65], 1.0)
    for j in range(8):
        ps = ps_pr.tile([128, 512], F32)
        for s in range(4):
            nc.tensor.matmul(ps[:, 0:64], x1T[:, s, j * 128:(j + 1) * 128],
                             Wkvb[:, s, 64:128], start=(s == 0), stop=(s == 3))
        nc.vector.tensor_add(out=vp[:, j, 0:64], in0=ps[:, 0:64], in1=tvb[:, 0:64])

    # q0^T both fine heads stacked [128, 4096] (head h on rows h*64..)
    q01T = sb.tile([128, N0], BF16)
    for c in range(8):
        ps = ps_pr.tile([128, 512], F32)
        for s in range(2):
            nc.tensor.matmul(ps[:], Wq0b[:, s, :],
                             x0T[:, s, c * 512:(c + 1) * 512],
                             start=(s == 0), stop=(s == 1))
        nc.vector.tensor_scalar(out=q01T[:, c * 512:(c + 1) * 512],
                                in0=ps[:], scalar1=tq0T[:], scalar2=0.0,
                                op0=ALU.add, op1=ALU.add)

    # q2^T coarse head pairs [128, 4, 256] (pair mt: head 2mt rows 0-63,
    # head 2mt+1 rows 64-127)
    q2T = sb.tile([128, 4, N2], BF16)
    for mt in range(4):
        ps = ps_pr.tile([128, 512], F32)
        for s in range(8):
            nc.tensor.matmul(ps[:, 0:256], Wq2b[:, s, mt * 128:(mt + 1) * 128],
                             x2T[:, s, :], start=(s == 0), stop=(s == 7))
        nc.vector.tensor_scalar(out=q2T[:, mt, :], in0=ps[:, 0:256],
                                scalar1=tq2T[:, mt:mt + 1], scalar2=0.0,
                                op0=ALU.add, op1=ALU.add)

    if DEBUG:
        nc.sync.dma_start(t["dbg_kT"][:], kTd[0:64, :])
        nc.sync.dma_start(t["dbg_vp"][:], vp[:])
        nc.sync.dma_start(t["dbg_q0T0"][:], q01T[0:64, :])
        nc.sync.dma_start(t["dbg_q2T"][:], q2T[:])
        nc.sync.dma_start(t["dbg_tq0"][:], tq0T[:])
        nc.sync.dma_start(t["dbg_tvb"][:], tvb[:])

    outs = sb.tile([128, 5, NK], BF16)
    inv_dram = dram.tile([1, 2 * N0 + H2 * N2], F32)

    # ---------------- fine attention ----------------
    # Both heads processed together: score matmuls row-packed (head 0 in PE
    # rows 0-63, head 1 in rows 64-127, concurrent), exp over a bank pair
    # holding both heads' scores for one t-subtile.
    oraw2h = [orawp.tile([65, 8, 512], F32, tag="oraw", name=f"oraw{h}")
              for h in range(2)]
    for c in range(8):
        expT = expp.tile([128, 8, 1024], BF16, tag="expT")
        for j in range(8):
            sps = ps_sc.tile([128, 1024], F32)
            for hh in range(2):
                nc.tensor.matmul(
                    sps[:, hh * 512:(hh + 1) * 512],
                    kTd[hh * 64:(hh + 1) * 64, j * 128:(j + 1) * 128],
                    q01T[hh * 64:(hh + 1) * 64, c * 512:(c + 1) * 512],
                    start=True, stop=True)
            nc.scalar.activation(out=expT[:, j, :], in_=sps[:], func=AF.Exp)
        for h in range(2):
            ops = ps_o.tile([65, 512], F32, tag="ops")
            for j in range(8):
                nc.tensor.matmul(ops[:], vp[:, j, :],
                                 expT[:, j, h * 512:(h + 1) * 512],
                                 start=(j == 0), stop=(j == 7))
            nc.vector.tensor_copy(oraw2h[h][:, c, :], ops[:])
    for h in range(2):
        oraw = oraw2h[h]
        if DEBUG and h == 0:
            nc.sync.dma_start(t["dbg_oraw0"][:], oraw[:])
        # raw denominators (row 64) -> DRAM -> broadcast -> reciprocal
        for c in range(8):
            nc.sync.dma_start(inv_dram[:, h * N0 + c * 512:h * N0 + (c + 1) * 512],
                              oraw[64:65, c, :])
        for c in range(8):
            invb = stg.tile([64, 512], F32, tag="invb")
            src = inv_dram[:, h * N0 + c * 512:h * N0 + (c + 1) * 512]
            nc.sync.dma_start(
                invb[:], bass.AP(tensor=src.tensor, offset=src.offset,
                                 ap=[[0, 64]] + list(src.ap[1:])))
            nc.vector.reciprocal_approx_fast(out=invb[:], in_=invb[:])
            if DEBUG and h == 0 and c == 0:
                nc.sync.dma_start(t["dbg_invb0"][:], invb[:])
            stage = stg.tile([64, 512], BF16, tag="stage")
            nc.vector.scalar_tensor_tensor(out=stage[:], in0=oraw[0:64, c, :],
                                           scalar=1.0, in1=invb[:],
                                           op0=ALU.mult, op1=ALU.mult)
            n1 = c // 2
            f0 = h * 320 + n1 * 64
            s0, p0 = f0 // 128, f0 % 128
            col = (c % 2) * 512
            nc.sync.dma_start(outs[p0:p0 + 64, s0, col:col + 512], stage[:])

    # ---------------- coarse attention ----------------
    # head pairs (2mt, 2mt+1) row-packed; psum tile holds 2 t-subtiles x 2
    # heads of scores: cols = u*512 + hh*256
    o2raw = orawp.tile([65, 8, 256], F32, tag="oraw2")
    for mt in range(4):
        # expT2 layout [128, hh, j, 256]; psum cols = hh*512 + u*256 so each
        # head's bank holds only its own (sequential) score groups
        expT2 = expp.tile([128, 2, 8, N2], BF16, tag="expT2")
        for jj in range(4):
            sps = ps_sc.tile([128, 1024], F32)
            for u in range(2):
                j = 2 * jj + u
                for hh in range(2):
                    nc.tensor.matmul(
                        sps[:, hh * 512 + u * 256:hh * 512 + (u + 1) * 256],
                        kTd[hh * 64:(hh + 1) * 64, j * 128:(j + 1) * 128],
                        q2T[hh * 64:(hh + 1) * 64, mt, :],
                        start=True, stop=True)
            nc.scalar.activation(out=expT2[:, :, 2 * jj:2 * jj + 2, :],
                                 in_=sps[:], func=AF.Exp)
        for hh in range(2):
            H = 2 * mt + hh
            ops = ps_o.tile([65, 512], F32, tag="ops")
            for j in range(8):
                nc.tensor.matmul(ops[:, 0:256], vp[:, j, :],
                                 expT2[:, hh, j, :],
                                 start=(j == 0), stop=(j == 7))
            nc.vector.tensor_copy(o2raw[:, H, :], ops[:, 0:256])
    base2 = 2 * N0
    for H in range(8):
        nc.sync.dma_start(inv_dram[:, base2 + H * 256:base2 + (H + 1) * 256],
                          o2raw[64:65, H, :])
    for H in range(8):
        invb = stg.tile([64, 512], F32, tag="invb")
        src = inv_dram[:, base2 + H * 256:base2 + (H + 1) * 256]
        nc.sync.dma_start(
            invb[0:64, 0:256], bass.AP(tensor=src.tensor, offset=src.offset,
                                       ap=[[0, 64]] + list(src.ap[1:])))
        nc.vector.reciprocal_approx_fast(out=invb[0:64, 0:256],
                                         in_=invb[0:64, 0:256])
        stage = stg.tile([64, 512], BF16, tag="stage")
        nc.vector.scalar_tensor_tensor(out=stage[:, 0:256],
                                       in0=o2raw[0:64, H, :], scalar=1.0,
                                       in1=invb[0:64, 0:256],
                                       op0=ALU.mult, op1=ALU.mult)
        h, h1 = H // 4, H % 4
        f0 = h * 320 + 256
        s0, p0 = f0 // 128, f0 % 128
        nc.sync.dma_start(outs[p0:p0 + 64, s0, h1 * 256:(h1 + 1) * 256],
                          stage[:, 0:256])

    if DEBUG:
        nc.sync.dma_start(t["dbg_outs"][:], outs[:])

    # ---------------- final projection ----------------
    for nt in range(8):
        ps = ps_pr.tile([128, 512], F32)
        for s in range(5):
            nc.tensor.matmul(ps[:], outs[:, s, nt * 128:(nt + 1) * 128],
                             Woutbf[:, s, :], start=(s == 0), stop=(s == 4))
        fo = outp.tile([128, 512], F32)
        nc.vector.tensor_copy(fo[:], ps[:])
        nc.sync.dma_start(t["out"][nt * 128:(nt + 1) * 128, :], fo[:])

    ctx.close()


_BUILT = None


def _build():
    global _BUILT
    if _BUILT is not None:
        return _BUILT
    nc = bacc.Bacc("TRN2", target_bir_lowering=False, debug=False)
    t = {
        "x0": nc.dram_tensor("x0", [N0, D0], F32, kind="ExternalInput"),
        "x1": nc.dram_tensor("x1", [NK, D1], F32, kind="ExternalInput"),
        "x2": nc.dram_tensor("x2", [N2, D2], F32, kind="ExternalInput"),
        "g0": nc.dram_tensor("g0", [D0], F32, kind="ExternalInput"),
        "b0": nc.dram_tensor("b0", [D0], F32, kind="ExternalInput"),
        "g1": nc.dram_tensor("g1", [D1], F32, kind="ExternalInput"),
        "b1": nc.dram_tensor("b1", [D1], F32, kind="ExternalInput"),
        "g2": nc.dram_tensor("g2", [D2], F32, kind="ExternalInput"),
        "b2": nc.dram_tensor("b2", [D2], F32, kind="ExternalInput"),
        "Wq0": nc.dram_tensor("Wq0", [D0, H0 * DH], F32, kind="ExternalInput"),
        "Wkv": nc.dram_tensor("Wkv", [D1, 2 * DH], F32, kind="ExternalInput"),
        "Wq2": nc.dram_tensor("Wq2", [D2, H2 * DH], F32, kind="ExternalInput"),
        "Wout": nc.dram_tensor("Wout", [CD, E], F32, kind="ExternalInput"),
        "out": nc.dram_tensor("out", [NK, E], F32, kind="ExternalOutput"),
    }
    if DEBUG:
        t.update({
            "dbg_x0T": nc.dram_tensor("dbg_x0T", [128, 2, N0], BF16, kind="ExternalOutput"),
            "dbg_x1T": nc.dram_tensor("dbg_x1T", [128, 4, NK], BF16, kind="ExternalOutput"),
            "dbg_kT": nc.dram_tensor("dbg_kT", [64, NK], BF16, kind="ExternalOutput"),
            "dbg_vp": nc.dram_tensor("dbg_vp", [128, 8, 65], BF16, kind="ExternalOutput"),
            "dbg_q0T0": nc.dram_tensor("dbg_q0T0", [64, N0], BF16, kind="ExternalOutput"),
            "dbg_q2T": nc.dram_tensor("dbg_q2T", [128, 4, N2], BF16, kind="ExternalOutput"),
            "dbg_tq0": nc.dram_tensor("dbg_tq0", [128, 1], F32, kind="ExternalOutput"),
            "dbg_tvb": nc.dram_tensor("dbg_tvb", [128, 64], F32, kind="ExternalOutput"),
            "dbg_oraw0": nc.dram_tensor("dbg_oraw0", [65, 8, 512], F32, kind="ExternalOutput"),
            "dbg_invb0": nc.dram_tensor("dbg_invb0", [64, 512], F32, kind="ExternalOutput"),
            "dbg_outs": nc.dram_tensor("dbg_outs", [128, 5, NK], BF16, kind="ExternalOutput"),
        })
    with tile.TileContext(nc) as tc:
        _emit(nc, tc, t)
    nc.compile()
    _BUILT = nc
    return nc


def kernel(**inputs):
    nc = _build()
    shared = {k: np.ascontiguousarray(np.asarray(inputs[k], dtype=np.float32))
              for k in ["g0", "b0", "g1", "b1", "g2", "b2",
                        "Wq0", "Wkv", "Wq2", "Wout"]}
    in_maps = []
    for b in range(B):
        m = dict(shared)
        m["x0"] = np.ascontiguousarray(np.asarray(inputs["x0"][b], np.float32))
        m["x1"] = np.ascontiguousarray(np.asarray(inputs["x1"][b], np.float32))
        m["x2"] = np.ascontiguousarray(np.asarray(inputs["x2"][b], np.float32))
        in_maps.append(m)
    res = bass_utils.run_bass_kernel_spmd(nc, in_maps, core_ids=list(range(B)))
    return np.stack([res.results[b]["out"] for b in range(B)], axis=0)


# revision 27
# speedup vs baseline: 126.6122x; 126.6122x over previous
"""Trainium2 Bass kernel for nn_Attention3 (dense multi-scale attention).

Sharding: pure data-parallel over batch — B=8, one batch element per
NeuronCore.  Each core runs the full per-batch computation; no collectives.

Per-core pipeline (all matmuls bf16 with fp32 PSUM accumulation):
  1. LayerNorm x0/x1/x2 in natural layout (DVE bn_stats/bn_aggr, ACT ln/exp
     for rsqrt), writing normalized x-hat as bf16 to DRAM scratch.
  2. xbar DMA-transpose x-hat back to SBUF in [d, n] layout.
  3. Projections q0^T / k^T / v / q2^T on PE (LN gamma/scale folded into
     weights, beta folded into per-channel bias columns).
  4. Attention in transposed layout: scores^T = k^T-vs-q^T matmuls,
     exp on ACT straight out of PSUM, o^T = v'-vs-exp^T matmuls where v'
     carries an extra all-ones column producing the softmax denominator.
  5. Normalize via reciprocal + DRAM-roundtrip row broadcast, assemble
     concat outs^T via partition-shifting SBUF DMAs, final out = outs @ Wout.

Fine-branch queries are loaded in (n1, n)-permuted order so the 4-token
channel fold becomes contiguous block copies.
"""
import os
import sys

sys.path.insert(0, "/opt/trn_rl_repo")

import numpy as np

DEBUG = bool(int(os.environ.get("K_DEBUG", "0")))

import concourse.bass as bass
import concourse.mybir as mybir
import concourse.tile as tile
from concourse import bacc, bass_utils

F32 = mybir.dt.float32
BF16 = mybir.dt.bfloat16
AF = mybir.ActivationFunctionType
ALU = mybir.AluOpType

B = 8
NK = 1024          # kv tokens
N0 = 4 * NK        # fine query tokens (4096)
N2 = NK // 4       # coarse query tokens (256)
D0, D1, D2 = 256, 512, 1024
DH = 64
H0, H2 = 2, 8
E = 512            # output dim
CD = 640           # concat dim
EPS = 1e-5
SCALE = DH ** -0.5


def _emit(nc, tc, t):
    """Emit the whole per-core kernel. t = dict of dram tensor handles."""
    import contextlib
    ctx = contextlib.ExitStack()

    sb = ctx.enter_context(tc.tile_pool(name="sb", bufs=1))
    ln = ctx.enter_context(tc.tile_pool(name="ln", bufs=2))
    wtmp = ctx.enter_context(tc.tile_pool(name="wtmp", bufs=2))
    expp = ctx.enter_context(tc.tile_pool(name="expp", bufs=2))
    orawp = ctx.enter_context(tc.tile_pool(name="orawp", bufs=2))
    stg = ctx.enter_context(tc.tile_pool(name="stg", bufs=2))
    outp = ctx.enter_context(tc.tile_pool(name="outp", bufs=2))
    ps_sc = ctx.enter_context(tc.tile_pool(name="ps_sc", bufs=2, space="PSUM"))
    ps_o = ctx.enter_context(tc.tile_pool(name="ps_o", bufs=2, space="PSUM"))
    ps_pr = ctx.enter_context(tc.tile_pool(name="ps_pr", bufs=2, space="PSUM"))
    dram = ctx.enter_context(tc.tile_pool(name="dram", bufs=1, space="DRAM"))

    # ---------------- weights prep ----------------
    # W stored as [128, S, Eout] with contraction dim striped over partitions.
    Wq0f = sb.tile([128, 2, 128], F32)
    nc.sync.dma_start(Wq0f[:], t["Wq0"][:].rearrange("(s p) e -> p s e", p=128))
    Wkvf = sb.tile([128, 4, 128], F32)
    nc.sync.dma_start(Wkvf[:], t["Wkv"][:].rearrange("(s p) e -> p s e", p=128))
    Woutbf = sb.tile([128, 5, 512], BF16)
    nc.gpsimd.dma_start(Woutbf[:], t["Wout"][:].rearrange("(s p) e -> p s e", p=128))

    g0 = sb.tile([128, 2], F32)
    nc.sync.dma_start(g0[:], t["g0"][:].rearrange("(s p) -> p s", p=128))
    b0 = sb.tile([128, 2], F32)
    nc.sync.dma_start(b0[:], t["b0"][:].rearrange("(s p) -> p s", p=128))
    g1 = sb.tile([128, 4], F32)
    nc.sync.dma_start(g1[:], t["g1"][:].rearrange("(s p) -> p s", p=128))
    b1 = sb.tile([128, 4], F32)
    nc.sync.dma_start(b1[:], t["b1"][:].rearrange("(s p) -> p s", p=128))
    g2 = sb.tile([128, 8], F32)
    nc.sync.dma_start(g2[:], t["g2"][:].rearrange("(s p) -> p s", p=128))
    b2 = sb.tile([128, 8], F32)
    nc.sync.dma_start(b2[:], t["b2"][:].rearrange("(s p) -> p s", p=128))

    # bias columns t^T = (b @ W)^T, computed before gamma folding
    tq0T = sb.tile([128, 1], F32)
    ps = ps_pr.tile([128, 512], F32)
    for s in range(2):
        nc.tensor.matmul(ps[:, 0:1], Wq0f[:, s, :], b0[:, s, None],
                         start=(s == 0), stop=(s == 1))
    nc.vector.tensor_scalar_mul(tq0T[:], ps[:, 0:1], SCALE)

    tkvT = sb.tile([128, 1], F32)
    ps = ps_pr.tile([128, 512], F32)
    for s in range(4):
        nc.tensor.matmul(ps[:, 0:1], Wkvf[:, s, :], b1[:, s, None],
                         start=(s == 0), stop=(s == 3))
    nc.vector.tensor_copy(tkvT[:], ps[:, 0:1])

    # t_v as a broadcast row: bias for v channels, [1,64] -> DRAM -> [128,64]
    tvrow = sb.tile([1, 64], F32)
    ps = ps_pr.tile([128, 512], F32)
    for s in range(4):
        nc.tensor.matmul(ps[0:1, 0:64], b1[:, s, None], Wkvf[:, s, 64:128],
                         start=(s == 0), stop=(s == 3))
    nc.vector.tensor_copy(tvrow[:], ps[0:1, 0:64])
    tv_dram = dram.tile([1, 64], F32)
    nc.sync.dma_start(tv_dram[:], tvrow[:])
    tvb = sb.tile([128, 64], F32)
    nc.sync.dma_start(
        tvb[:],
        bass.AP(tensor=tv_dram.tensor, offset=tv_dram[:].offset,
                ap=[[0, 128]] + list(tv_dram[:].ap[1:])),
    )

    # gamma (+ attention scale) folded into bf16 weights
    Wq0b = sb.tile([128, 2, 128], BF16)
    for s in range(2):
        nc.vector.tensor_scalar(out=Wq0b[:, s, :], in0=Wq0f[:, s, :],
                                scalar1=g0[:, s, None], scalar2=SCALE,
                                op0=ALU.mult, op1=ALU.mult)
    Wkvb = sb.tile([128, 4, 128], BF16)
    for s in range(4):
        nc.vector.tensor_scalar(out=Wkvb[:, s, :], in0=Wkvf[:, s, :],
                                scalar1=g1[:, s, None], scalar2=1.0,
                                op0=ALU.mult, op1=ALU.mult)

    # Wq2 streamed per K-slice (16KB/partition is too much to keep in f32).
    # t_q2 computed in row form [1, 512] (single PSUM group), then converted
    # to column form [128, 4] via a DRAM roundtrip.
    Wq2b = sb.tile([128, 8, 512], BF16)
    ps_b = ps_pr.tile([128, 512], F32, tag="ps", name="ps_b")
    for s in range(8):
        wq2s = wtmp.tile([128, 512], F32)
        nc.sync.dma_start(
            wq2s[:], t["Wq2"][:].rearrange("(s p) e -> p s e", p=128)[:, s, :])
        nc.tensor.matmul(ps_b[0:1, :], b2[:, s, None], wq2s[:],
                         start=(s == 0), stop=(s == 7))
        nc.vector.tensor_scalar(out=Wq2b[:, s, :], in0=wq2s[:],
                                scalar1=g2[:, s, None], scalar2=SCALE,
                                op0=ALU.mult, op1=ALU.mult)
    tq2row = sb.tile([1, 512], F32)
    nc.vector.tensor_scalar_mul(tq2row[:], ps_b[0:1, :], SCALE)
    tq2_dram = dram.tile([1, 512], F32)
    nc.sync.dma_start(tq2_dram[:], tq2row[:])
    tq2T = sb.tile([128, 4], F32)
    nc.sync.dma_start(tq2T[:], tq2_dram[0, :].rearrange("(mt p) -> p mt", p=128))

    # ---------------- LayerNorm + transpose ----------------
    eps_t = sb.tile([128, 1], F32)
    nc.vector.memset(eps_t[:], EPS)

    def emit_ln(name, xap, n_tokens, d, perm):
        """LN in natural layout (bf16 cast-load) -> bf16 x-hat in DRAM ->
        transposed SBUF.  Transposes run per token-group so projections can
        start before the whole tensor is normalized."""
        ntiles = n_tokens // 128
        nsub = max(1, d // 512)
        tile_groups = max(1, 2048 // d)  # tiles per group (group = 2048 elems)
        ngroups = ntiles // tile_groups
        grows = tile_groups * 128       # token rows per group
        xhat_d = dram.tile([n_tokens, d], BF16)
        if perm:
            src = xap.rearrange("(n n1) d -> n1 n d", n1=4)
        else:
            src = xap.rearrange("(g n) d -> g n d", g=ngroups)
        xT = sb.tile([128, d // 128, n_tokens], BF16, tag=f"xT_{name}",
                     name=f"xT_{name}")
        for g in range(ngroups):
            xf = ln.tile([128, tile_groups, d], BF16, tag="lnxf")
            # one cast-DMA per group; for perm the rows are strided by 4
            gsrc = src[g].rearrange("(tl p) d -> p tl d", p=128)
            nc.gpsimd.dma_start(xf[:], gsrc)
            stats = ln.tile([128, tile_groups, nsub, 6], F32, tag="lnstats")
            mv = ln.tile([128, tile_groups, 2], F32, tag="lnmv")
            for i in range(tile_groups):
                for k in range(nsub):
                    nc.vector.bn_stats(stats[:, i, k, :],
                                       xf[:, i, k * 512:(k + 1) * 512] if nsub > 1
                                       else xf[:, i, :])
                nc.vector.bn_aggr(mv[:, i, :], stats[:, i, :, :])
            rs = ln.tile([128, tile_groups], F32, tag="lnrs")
            nc.scalar.activation(out=rs[:], in_=mv[:, :, 1],
                                 func=AF.Abs_reciprocal_sqrt,
                                 bias=eps_t[:], scale=1.0)
            xh = ln.tile([128, tile_groups, d], BF16, tag="lnxh")
            for i in range(tile_groups):
                nc.vector.tensor_scalar(out=xh[:, i, :], in0=xf[:, i, :],
                                        scalar1=mv[:, i, 0:1],
                                        scalar2=rs[:, i, None],
                                        op0=ALU.subtract, op1=ALU.mult)
            dst = xhat_d[:].rearrange("(g tl p) d -> g p tl d", g=ngroups, p=128)
            nc.sync.dma_start(dst[g], xh[:])
            # transpose this group's rows: [grows, 128] -> [128, grows]
            for s in range(d // 128):
                nc.sync.dma_start_transpose(
                    xT[:, s, g * grows:(g + 1) * grows],
                    xhat_d[g * grows:(g + 1) * grows, s * 128:(s + 1) * 128])
        return xT

    x1T = emit_ln("x1", t["x1"][:], NK, D1, perm=False)
    x0T = emit_ln("x0", t["x0"][:], N0, D0, perm=True)
    x2T = emit_ln("x2", t["x2"][:], N2, D2, perm=False)
    if DEBUG:
        nc.sync.dma_start(t["dbg_x0T"][:], x0T[:])
        nc.sync.dma_start(t["dbg_x1T"][:], x1T[:])

    # ---------------- projections ----------------
    # k^T duplicated on both partition halves [128, 1024] for row-packed
    # head-pair score matmuls (rows 0-63 == rows 64-127 == k^T)
    kTd = sb.tile([128, NK], BF16)
    for half in range(2):
        ps = ps_pr.tile([128, 512], F32)
        for s in range(4):
            nc.tensor.matmul(ps[0:64, :], Wkvb[:, s, 0:64],
                             x1T[:, s, half * 512:(half + 1) * 512],
                             start=(s == 0), stop=(s == 3))
        nc.vector.tensor_scalar(out=kTd[0:64, half * 512:(half + 1) * 512],
                                in0=ps[0:64, :], scalar1=tkvT[0:64, :],
                                scalar2=0.0, op0=ALU.add, op1=ALU.add)
        nc.vector.tensor_scalar(out=kTd[64:128, half * 512:(half + 1) * 512],
                                in0=ps[0:64, :], scalar1=tkvT[0:64, :],
                                scalar2=0.0, op0=ALU.add, op1=ALU.add)

    # v' natural [128(t), 8, 65]: 64 v channels + ones column (denominator)
    vp = sb.tile([128, 8, 65], BF16)
    nc.vector.memset(vp[:, :, 64:65], 1.0)
    for j in range(8):
        ps = ps_pr.tile([128, 512], F32)
        for s in range(4):
            nc.tensor.matmul(ps[:, 0:64], x1T[:, s, j * 128:(j + 1) * 128],
                             Wkvb[:, s, 64:128], start=(s == 0), stop=(s == 3))
        nc.vector.tensor_add(out=vp[:, j, 0:64], in0=ps[:, 0:64], in1=tvb[:, 0:64])

    # q0^T both fine heads stacked [128, 4096] (head h on rows h*64..)
    q01T = sb.tile([128, N0], BF16)
    for c in range(8):
        ps = ps_pr.tile([128, 512], F32)
        for s in range(2):
            nc.tensor.matmul(ps[:], Wq0b[:, s, :],
                             x0T[:, s, c * 512:(c + 1) * 512],
                             start=(s == 0), stop=(s == 1))
        nc.vector.tensor_scalar(out=q01T[:, c * 512:(c + 1) * 512],
                                in0=ps[:], scalar1=tq0T[:], scalar2=0.0,
                                op0=ALU.add, op1=ALU.add)

    # q2^T coarse head pairs [128, 4, 256] (pair mt: head 2mt rows 0-63,
    # head 2mt+1 rows 64-127)
    q2T = sb.tile([128, 4, N2], BF16)
    for mt in range(4):
        ps = ps_pr.tile([128, 512], F32)
        for s in range(8):
            nc.tensor.matmul(ps[:, 0:256], Wq2b[:, s, mt * 128:(mt + 1) * 128],
                             x2T[:, s, :], start=(s == 0), stop=(s == 7))
        nc.vector.tensor_scalar(out=q2T[:, mt, :], in0=ps[:, 0:256],
                                scalar1=tq2T[:, mt:mt + 1], scalar2=0.0,
                                op0=ALU.add, op1=ALU.add)

    if DEBUG:
        nc.sync.dma_start(t["dbg_kT"][:], kTd[0:64, :])
        nc.sync.dma_start(t["dbg_vp"][:], vp[:])
        nc.sync.dma_start(t["dbg_q0T0"][:], q01T[0:64, :])
        nc.sync.dma_start(t["dbg_q2T"][:], q2T[:])
        nc.sync.dma_start(t["dbg_tq0"][:], tq0T[:])
        nc.sync.dma_start(t["dbg_tvb"][:], tvb[:])

    outs = sb.tile([128, 5, NK], BF16)
    inv_dram = dram.tile([1, 2 * N0 + H2 * N2], F32)

    # ---------------- fine attention ----------------
    # Both heads processed together: score matmuls row-packed (head 0 in PE
    # rows 0-63, head 1 in rows 64-127, concurrent), exp over a bank pair
    # holding both heads' scores for one t-subtile.
    oraw2h = [orawp.tile([65, 8, 512], F32, tag="oraw", name=f"oraw{h}")
              for h in range(2)]
    for c in range(8):
        expT = expp.tile([128, 8, 1024], BF16, tag="expT")
        for j in range(8):
            sps = ps_sc.tile([128, 1024], F32)
            for hh in range(2):
                nc.tensor.matmul(
                    sps[:, hh * 512:(hh + 1) * 512],
                    kTd[hh * 64:(hh + 1) * 64, j * 128:(j + 1) * 128],
                    q01T[hh * 64:(hh + 1) * 64, c * 512:(c + 1) * 512],
                    start=True, stop=True)
            nc.scalar.activation(out=expT[:, j, :], in_=sps[:], func=AF.Exp)
        for h in range(2):
            ops = ps_o.tile([65, 512], F32, tag="ops")
            for j in range(8):
                nc.tensor.matmul(ops[:], vp[:, j, :],
                                 expT[:, j, h * 512:(h + 1) * 512],
                                 start=(j == 0), stop=(j == 7))
            nc.vector.tensor_copy(oraw2h[h][:, c, :], ops[:])
    for h in range(2):
        oraw = oraw2h[h]
        if DEBUG and h == 0:
            nc.sync.dma_start(t["dbg_oraw0"][:], oraw[:])
        # raw denominators (row 64) -> DRAM -> broadcast -> reciprocal
        for c in range(8):
            nc.sync.dma_start(inv_dram[:, h * N0 + c * 512:h * N0 + (c + 1) * 512],
                              oraw[64:65, c, :])
        for c in range(8):
            invb = stg.tile([64, 512], F32, tag="invb")
            src = inv_dram[:, h * N0 + c * 512:h * N0 + (c + 1) * 512]
            nc.sync.dma_start(
                invb[:], bass.AP(tensor=src.tensor, offset=src.offset,
                                 ap=[[0, 64]] + list(src.ap[1:])))
            nc.vector.reciprocal_approx_fast(out=invb[:], in_=invb[:])
            if DEBUG and h == 0 and c == 0:
                nc.sync.dma_start(t["dbg_invb0"][:], invb[:])
            stage = stg.tile([64, 512], BF16, tag="stage")
            nc.vector.scalar_tensor_tensor(out=stage[:], in0=oraw[0:64, c, :],
                                           scalar=1.0, in1=invb[:],
                                           op0=ALU.mult, op1=ALU.mult)
            n1 = c // 2
            f0 = h * 320 + n1 * 64
            s0, p0 = f0 // 128, f0 % 128
            col = (c % 2) * 512
            nc.sync.dma_start(outs[p0:p0 + 64, s0, col:col + 512], stage[:])

    # ---------------- coarse attention ----------------
    # head pairs (2mt, 2mt+1) row-packed; psum tile holds 2 t-subtiles x 2
    # heads of scores: cols = u*512 + hh*256
    o2raw = orawp.tile([65, 8, 256], F32, tag="oraw2")
    for mt in range(4):
        # expT2 layout [128, hh, j, 256]; psum cols = hh*512 + u*256 so each
        # head's bank holds only its own (sequential) score groups
        expT2 = expp.tile([128, 2, 8, N2], BF16, tag="expT2")
        for jj in range(4):
            sps = ps_sc.tile([128, 1024], F32)
            for u in range(2):
                j = 2 * jj + u
                for hh in range(2):
                    nc.tensor.matmul(
                        sps[:, hh * 512 + u * 256:hh * 512 + (u + 1) * 256],
                        kTd[hh * 64:(hh + 1) * 64, j * 128:(j + 1) * 128],
                        q2T[hh * 64:(hh + 1) * 64, mt, :],
                        start=True, stop=True)
            nc.scalar.activation(out=expT2[:, :, 2 * jj:2 * jj + 2, :],
                                 in_=sps[:], func=AF.Exp)
        for hh in range(2):
            H = 2 * mt + hh
            ops = ps_o.tile([65, 512], F32, tag="ops")
            for j in range(8):
                nc.tensor.matmul(ops[:, 0:256], vp[:, j, :],
                                 expT2[:, hh, j, :],
                                 start=(j == 0), stop=(j == 7))
            nc.vector.tensor_copy(o2raw[:, H, :], ops[:, 0:256])
    base2 = 2 * N0
    for H in range(8):
        nc.sync.dma_start(inv_dram[:, base2 + H * 256:base2 + (H + 1) * 256],
                          o2raw[64:65, H, :])
    for H in range(8):
        invb = stg.tile([64, 512], F32, tag="invb")
        src = inv_dram[:, base2 + H * 256:base2 + (H + 1) * 256]
        nc.sync.dma_start(
            invb[0:64, 0:256], bass.AP(tensor=src.tensor, offset=src.offset,
                                       ap=[[0, 64]] + list(src.ap[1:])))
        nc.vector.reciprocal_approx_fast(out=invb[0:64, 0:256],
                                         in_=invb[0:64, 0:256])
        stage = stg.tile([64, 512], BF16, tag="stage")
        nc.vector.scalar_tensor_tensor(out=stage[:, 0:256],
                                       in0=o2raw[0:64, H, :], scalar=1.0,
                                       in1=invb[0:64, 0:256],
                                       op0=ALU.mult, op1=ALU.mult)
        h, h1 = H // 4, H % 4
        f0 = h * 320 + 256
        s0, p0 = f0 // 128, f0 % 128
        nc.sync.dma_start(outs[p0:p0 + 64, s0, h1 * 256:(h1 + 1) * 256],
                          stage[:, 0:256])

    if DEBUG:
        nc.sync.dma_start(t["dbg_outs"][:], outs[:])

    # ---------------- final projection ----------------
    for nt in range(8):
        ps = ps_pr.tile([128, 512], F32)
        for s in range(5):
            nc.tensor.matmul(ps[:], outs[:, s, nt * 128:(nt + 1) * 128],
                             Woutbf[:, s, :], start=(s == 0), stop=(s == 4))
        fo = outp.tile([128, 512], F32)
        nc.vector.tensor_copy(fo[:], ps[:])
        nc.sync.dma_start(t["out"][nt * 128:(nt + 1) * 128, :], fo[:])

    ctx.close()


_BUILT = None


def _build():
    global _BUILT
    if _BUILT is not None:
        return _BUILT
    nc = bacc.Bacc("TRN2", target_bir_lowering=False, debug=False)
    t = {
        "x0": nc.dram_tensor("x0", [N0, D0], F32, kind="ExternalInput"),
        "x1": nc.dram_tensor("x1", [NK, D1], F32, kind="ExternalInput"),
        "x2": nc.dram_tensor("x2", [N2, D2], F32, kind="ExternalInput"),
        "g0": nc.dram_tensor("g0", [D0], F32, kind="ExternalInput"),
        "b0": nc.dram_tensor("b0", [D0], F32, kind="ExternalInput"),
        "g1": nc.dram_tensor("g1", [D1], F32, kind="ExternalInput"),
        "b1": nc.dram_tensor("b1", [D1], F32, kind="ExternalInput"),
        "g2": nc.dram_tensor("g2", [D2], F32, kind="ExternalInput"),
        "b2": nc.dram_tensor("b2", [D2], F32, kind="ExternalInput"),
        "Wq0": nc.dram_tensor("Wq0", [D0, H0 * DH], F32, kind="ExternalInput"),
        "Wkv": nc.dram_tensor("Wkv", [D1, 2 * DH], F32, kind="ExternalInput"),
        "Wq2": nc.dram_tensor("Wq2", [D2, H2 * DH], F32, kind="ExternalInput"),
        "Wout": nc.dram_tensor("Wout", [CD, E], F32, kind="ExternalInput"),
        "out": nc.dram_tensor("out", [NK, E], F32, kind="ExternalOutput"),
    }
    if DEBUG:
        t.update({
            "dbg_x0T": nc.dram_tensor("dbg_x0T", [128, 2, N0], BF16, kind="ExternalOutput"),
            "dbg_x1T": nc.dram_tensor("dbg_x1T", [128, 4, NK], BF16, kind="ExternalOutput"),
            "dbg_kT": nc.dram_tensor("dbg_kT", [64, NK], BF16, kind="ExternalOutput"),
            "dbg_vp": nc.dram_tensor("dbg_vp", [128, 8, 65], BF16, kind="ExternalOutput"),
            "dbg_q0T0": nc.dram_tensor("dbg_q0T0", [64, N0], BF16, kind="ExternalOutput"),
            "dbg_q2T": nc.dram_tensor("dbg_q2T", [128, 4, N2], BF16, kind="ExternalOutput"),
            "dbg_tq0": nc.dram_tensor("dbg_tq0", [128, 1], F32, kind="ExternalOutput"),
            "dbg_tvb": nc.dram_tensor("dbg_tvb", [128, 64], F32, kind="ExternalOutput"),
            "dbg_oraw0": nc.dram_tensor("dbg_oraw0", [65, 8, 512], F32, kind="ExternalOutput"),
            "dbg_invb0": nc.dram_tensor("dbg_invb0", [64, 512], F32, kind="ExternalOutput"),
            "dbg_outs": nc.dram_tensor("dbg_outs", [128, 5, NK], BF16, kind="ExternalOutput"),
        })
    iters = int(os.environ.get("K_ITERS", "1"))
    with tile.TileContext(nc) as tc:
        if iters > 1:
            with tc.For_i(0, iters, 1):
                _emit(nc, tc, t)
        else:
            _emit(nc, tc, t)
    nc.compile()
    _BUILT = nc
    return nc


def kernel(**inputs):
    nc = _build()
    shared = {k: np.ascontiguousarray(np.asarray(inputs[k], dtype=np.float32))
              for k in ["g0", "b0", "g1", "b1", "g2", "b2",
                        "Wq0", "Wkv", "Wq2", "Wout"]}
    in_maps = []
    for b in range(B):
        m = dict(shared)
        m["x0"] = np.ascontiguousarray(np.asarray(inputs["x0"][b], np.float32))
        m["x1"] = np.ascontiguousarray(np.asarray(inputs["x1"][b], np.float32))
        m["x2"] = np.ascontiguousarray(np.asarray(inputs["x2"][b], np.float32))
        in_maps.append(m)
    res = bass_utils.run_bass_kernel_spmd(nc, in_maps, core_ids=list(range(B)))
    return np.stack([res.results[b]["out"] for b in range(B)], axis=0)
